# revision 20
# baseline (speedup 1.0000x reference)
"""BERT-BiLSTM-CRF Trainium2 kernel (8 NeuronCores, SPMD).

Pipeline per core (2048 tokens + 96-token halo each side):
  1. DMA-transpose x slice (bf16) -> xT [768, 2304] in SBUF
  2. PE: gates = xT.T @ Wcat (i,g,o gates only, both LSTM directions) in bf16
  3. ACT/DVE: h = sigmoid(o) * tanh(sigmoid(i) * tanh(g))  -> enc (bf16)
  4. PE: feats[token, tag] = enc.T @ w_tag.T + b_tag  (bias via K=2 ones matmul)
  5. feats -> HBM roundtrip into chunk-major scan layouts
  6. DVE: chunked Viterbi scans (128 chunks x L=16, warmup W=96, 19-tag spaces
     interior+START fwd / interior+STOP bwd), zero-init warm start; exact init
     override for the global first (fwd) / last (bwd) chunk
  7. DVE: decode p_t = argmax_j(fv+gv-feat) over 18 interior tags + gather
     emission feat[t, p_t]
Host: path score by f32-faithful re-accumulation along the decoded path
(reference op order), so the returned score matches the reference's f32 scan.
"""
import sys
import numpy as np

sys.path.insert(0, "/opt/trn_rl_repo")

import ml_dtypes

bf16 = ml_dtypes.bfloat16

# problem constants (hardcoded per contract)
H = 768
HALF = 256
T = 20
START, STOP = 18, 19
S = 16386
N = 16384
NEG = -10000.0
NCORES = 8
NPC = N // NCORES          # 2048 tokens per core
PAD = 96                   # halo = warmup length
W = 96                     # warmup steps
L = 16                     # chunk length
NSTEP = W + L              # 112
T19 = 19
INTER = 18
LT = PAD + NPC + PAD       # 2240 local tokens
LTP = 2304                 # padded to 18*128 rows for GEMM
NCH = 128                  # chunks per core
TOKCH = 384                # token chunk for gates GEMM (6 chunks over 2304)

_CACHE = {}


def _build_nc():
    from concourse import bass, mybir
    from concourse.bass import AP

    dt = mybir.dt
    Alu = mybir.AluOpType
    Act = mybir.ActivationFunctionType

    nc = bass.Bass("TRN2", target_bir_lowering=False, debug=False)

    xb = nc.declare_dram_parameter("xb", [LTP, H], dt.bfloat16, isOutput=False)
    f32c = nc.declare_dram_parameter("f32c", [128, 792], dt.float32, isOutput=False)
    wbf = nc.declare_dram_parameter("wbf", [128, 9296], dt.bfloat16, isOutput=False)
    ob2 = nc.declare_dram_parameter("ob2", [2, 148], dt.bfloat16, isOutput=False)
    pout = nc.declare_dram_parameter("pout", [128, L], dt.int32, isOutput=True)
    femis_o = nc.declare_dram_parameter("femis", [128, L], dt.float32, isOutput=True)
    featsH = nc.dram_tensor("featsH", [LTP, T], dt.float32)

    KT = H // 128          # 6
    ET = 512 // 128        # 4
    NTOK = LTP // TOKCH    # 6
    TT_ = LTP // 128       # 18
    NTRI = NTOK * 4        # 24 triples total

    from contextlib import ExitStack
    ctx = ExitStack()
    sem = lambda n: ctx.enter_context(nc.semaphore(n))
    sb = lambda n, shp, d: ctx.enter_context(nc.sbuf_tensor(n, shp, d))
    psum = lambda n, shp: ctx.enter_context(nc.psum_tensor(n, shp, dt.float32))

    dS = sem("dS"); dH = sem("dH"); pS = sem("pS"); aB = sem("aB"); aT = sem("aT")
    vM = sem("vM"); fS = sem("fS"); vD = sem("vD")

    xT = [sb(f"xT{k}", [128, LTP], dt.bfloat16) for k in range(KT)]
    wbf_sb = sb("wbf_sb", [128, 9296], dt.bfloat16)
    f32c_sb = sb("f32c_sb", [128, 792], dt.float32)
    ob2_sb = sb("ob2_sb", [2, 148], dt.bfloat16)
    enc = [sb(f"enc{e}", [128, LTP], dt.bfloat16) for e in range(ET)]
    actb = {n: [sb(f"{n}{i}", [128, TOKCH], dt.bfloat16) for i in range(2)]
            for n in ("si", "tg", "so", "cc", "tc")}
    feats_dec = sb("feats_dec", [128, TT_ * T], dt.float32)
    fwf = sb("fwf", [128, NSTEP * T19], dt.float32)
    fwb = sb("fwb", [128, NSTEP * T19], dt.float32)
    fvstore = sb("fvstore", [128, L * T19], dt.float32)
    gvstore = sb("gvstore", [128, L * T19], dt.float32)
    wbF = sb("wbF", [128, T19], dt.float32)
    wbB = sb("wbB", [128, T19], dt.float32)
    tmpF = sb("tmpF", [128, T19 * T19], dt.float32)
    tmpB = sb("tmpB", [128, T19 * T19], dt.float32)
    redF = sb("redF", [128, T19], dt.float32)
    redB = sb("redB", [128, T19], dt.float32)
    decA = sb("decA", [128, L * INTER], dt.float32)
    decB = sb("decB", [128, L * INTER], dt.float32)
    decK = sb("decK", [128, L * INTER], dt.float32)
    mx = sb("mx", [128, L], dt.float32)
    pf = sb("pf", [128, L], dt.float32)
    pi = sb("pi", [128, L], dt.int32)
    fem = sb("fem", [128, L], dt.float32)

    gb = [psum(f"gb{i}", [128, 512]) for i in range(6)]      # gates banks (bank-padded)
    fb = [psum(f"fb{i}", [128, 512]) for i in range(2)]          # feats banks (bank-padded)

    def ap(h, shape2):
        return AP(h, 0, [[shape2[1], shape2[0]], [1, shape2[1]]])

    xT_a = [ap(h, (128, LTP)) for h in xT]
    wbf_a = ap(wbf_sb, (128, 9296))
    f32c_a = ap(f32c_sb, (128, 792))
    ob2_a = ap(ob2_sb, (2, 148))
    enc_a = [ap(h, (128, LTP)) for h in enc]
    actb_a = {n: [ap(t, (128, TOKCH)) for t in actb[n]] for n in actb}
    feats_dec_a = ap(feats_dec, (128, TT_ * T))
    fwf_a = ap(fwf, (128, NSTEP * T19))
    fwb_a = ap(fwb, (128, NSTEP * T19))
    fwf3 = fwf_a.rearrange("p (s j) -> p s j", j=T19)
    fwb3 = fwb_a.rearrange("p (s j) -> p s j", j=T19)
    fvs3 = ap(fvstore, (128, L * T19)).rearrange("p (s j) -> p s j", j=T19)
    gvs3 = ap(gvstore, (128, L * T19)).rearrange("p (s j) -> p s j", j=T19)
    wbF_a = ap(wbF, (128, T19)); wbB_a = ap(wbB, (128, T19))
    tmpF3 = ap(tmpF, (128, T19 * T19)).rearrange("p (j i) -> p j i", i=T19)
    tmpB3 = ap(tmpB, (128, T19 * T19)).rearrange("p (j i) -> p j i", i=T19)
    redF_a = ap(redF, (128, T19)); redB_a = ap(redB, (128, T19))
    A3 = ap(decA, (128, L * INTER)).rearrange("p (s j) -> p s j", j=INTER)
    B3 = ap(decB, (128, L * INTER)).rearrange("p (s j) -> p s j", j=INTER)
    K3 = ap(decK, (128, L * INTER)).rearrange("p (s j) -> p s j", j=INTER)
    mx_a = ap(mx, (128, L)); pf_a = ap(pf, (128, L))
    pi_a = ap(pi, (128, L)); fem_a = ap(fem, (128, L))
    gb_a = [ap(h, (128, 512))[:, 0:TOKCH] for h in gb]
    fb_a = [ap(h, (128, 512))[:, 0:T] for h in fb]

    w_sb = [wbf_a[:, k * 1536:(k + 1) * 1536] for k in range(KT)]
    wtag_sb = [wbf_a[:, 9216 + k * T: 9216 + (k + 1) * T] for k in range(ET)]
    trf3 = f32c_a[:, 0:361].rearrange("p (j i) -> p j i", i=T19)
    trb3 = f32c_a[:, 361:722].rearrange("p (j i) -> p j i", i=T19)
    maskf_sb = f32c_a[:, 722:723]
    initvf_sb = f32c_a[:, 723:742]
    maskb_sb = f32c_a[:, 742:743]
    initvb_sb = f32c_a[:, 743:762]
    iota_sb = f32c_a[:, 762:780]
    bcat_sb = f32c_a[:, 780:792]
    ones_sb = ob2_a[:, 0:128]
    btag_sb = ob2_a[:, 128:148]

    # triple t -> (chunk, gate tiles, enc row tile)
    triples = [(0, 2, 4, 0), (1, 3, 5, 1), (6, 8, 10, 2), (7, 9, 11, 3)]

    with nc.Block() as block:

        @block.sync
        def _(sync):
            for k in range(KT):
                sync.dma_start_transpose(
                    out=xT_a[k], in_=xb[:, k * 128:(k + 1) * 128]
                ).then_inc(dH, 16)

        @block.gpsimd
        def _(gp):
            # serialize after the xbar transposes (DMATranspose<->DMACopy hang)
            gp.wait_ge(dH, 96)
            gp.dma_start(out=f32c_sb[:], in_=f32c[:]).then_inc(dS, 16)
            gp.dma_start(out=wbf_sb[:], in_=wbf[:]).then_inc(dS, 16)
            gp.dma_start(out=ob2_sb[:], in_=ob2[:]).then_inc(dS, 16)
            # feats roundtrip
            gp.wait_ge(fS, TT_)
            fh_out = AP(featsH, 0, [[T, 128], [128 * T, TT_], [1, T]])
            gp.dma_start(
                out=fh_out,
                in_=feats_dec_a.rearrange("p (tt j) -> p tt j", j=T),
            ).then_inc(dS, 16)
            gp.wait_ge(dS, 64)
            src_f = AP(featsH, 0, [[16 * T, 128], [T, NSTEP], [1, T19]])
            gp.dma_start(out=fwf3, in_=src_f).then_inc(dS, 16)
            src_b = AP(featsH, PAD * T, [[16 * T, 128], [T, NSTEP], [1, INTER]])
            gp.dma_start(out=fwb3[:, :, 0:INTER], in_=src_b).then_inc(dS, 16)
            src_b2 = AP(featsH, PAD * T + STOP, [[16 * T, 128], [T, NSTEP], [1, 1]])
            with nc.allow_non_contiguous_dma(reason="112x1 stop-column gather"):
                gp.dma_start(out=fwb3[:, :, INTER:T19], in_=src_b2).then_inc(dS, 16)
            # outputs
            gp.wait_ge(vD, 1)
            gp.dma_start(out=pout[:], in_=pi_a).then_inc(dS, 16)
            gp.dma_start(out=femis_o[:], in_=fem_a).then_inc(dS, 16)
            gp.wait_ge(dS, 144)

        @block.tensor
        def _(te):
            te.wait_ge(dH, 96)
            te.wait_ge(dS, 48)
            for t in range(NTRI):
                c, tr = divmod(t, 4)
                gi, gg, go, eidx = triples[tr]
                tsl = slice(c * TOKCH, (c + 1) * TOKCH)
                if t >= 2:
                    te.wait_ge(aB, t - 1)
                for j, g in enumerate((gi, gg, go)):
                    bank = gb_a[(t % 2) * 3 + j]
                    for k in range(KT):
                        mm = te.matmul(
                            bank, w_sb[k][:, g * 128:(g + 1) * 128],
                            xT_a[k][:, tsl],
                            start=(k == 0), stop=(k == KT - 1),
                        )
                    mm.then_inc(pS, 1)
            for tt in range(TT_):
                c_hi = ((tt + 1) * 128 - 1) // TOKCH
                te.wait_ge(vM, 8 * (c_hi + 1))
                if tt >= 2:
                    te.wait_ge(fS, tt - 1)
                bank = fb_a[tt % 2]
                for k in range(ET):
                    te.matmul(bank, enc_a[k][:, tt * 128:(tt + 1) * 128],
                              wtag_sb[k], start=(k == 0), stop=False)
                te.matmul(bank, ones_sb, btag_sb,
                          start=False, stop=True).then_inc(pS, 1)

        @block.scalar
        def _(sc):
            sc.wait_ge(dS, 48)
            for t in range(NTRI):
                c, tr = divmod(t, 4)
                gi, gg, go, eidx = triples[tr]
                b = t % 2
                sc.wait_ge(pS, 3 * t + 3)
                if t >= 2:
                    sc.wait_ge(vM, 2 * (t - 2) + 2)
                sc.activation(actb_a["si"][b], gb_a[b * 3 + 0], Act.Sigmoid,
                              bias=bcat_sb[:, gi:gi + 1])
                sc.activation(actb_a["tg"][b], gb_a[b * 3 + 1], Act.Tanh,
                              bias=bcat_sb[:, gg:gg + 1])
                sc.activation(actb_a["so"][b], gb_a[b * 3 + 2], Act.Sigmoid,
                              bias=bcat_sb[:, go:go + 1]).then_inc(aB, 1)
                sc.wait_ge(vM, 2 * t + 1)
                sc.activation(actb_a["tc"][b], actb_a["cc"][b],
                              Act.Tanh).then_inc(aT, 1)
            for tt in range(TT_):
                sc.wait_ge(pS, 3 * NTRI + tt + 1)
                sc.activation(feats_dec_a[:, tt * T:(tt + 1) * T],
                              fb_a[tt % 2], Act.Copy).then_inc(fS, 1)

        @block.vector
        def _(v):
            for t in range(NTRI):
                c, tr = divmod(t, 4)
                gi, gg, go, eidx = triples[tr]
                b = t % 2
                tsl = slice(c * TOKCH, (c + 1) * TOKCH)
                v.wait_ge(aB, t + 1)
                v.tensor_tensor(out=actb_a["cc"][b], in0=actb_a["si"][b],
                                in1=actb_a["tg"][b], op=Alu.mult).then_inc(vM, 1)
                v.wait_ge(aT, t + 1)
                v.tensor_tensor(out=enc_a[eidx][:, tsl], in0=actb_a["so"][b],
                                in1=actb_a["tc"][b], op=Alu.mult).then_inc(vM, 1)

            v.wait_ge(dS, 112)

            def scan(tr3, fw3, store3, mask_ap, initv_ap, wb_a, tmp3, red_a,
                     backward):
                v.tensor_scalar_mul(wb_a, f32c_a[:, 0:T19], 0.0)
                v.drain()
                for s in range(NSTEP):
                    if s <= W:
                        src = wb_a
                    else:
                        psl = (NSTEP - s) if backward else (s - 1 - W)
                        src = store3[:, psl, :]
                    if s == W:
                        v.scalar_tensor_tensor(
                            out=wb_a, in0=wb_a, scalar=mask_ap,
                            in1=initv_ap, op0=Alu.mult, op1=Alu.add)
                        v.drain()
                        src = wb_a
                    fslot = (NSTEP - 1 - s) if backward else s
                    bsrc = src.unsqueeze(1).broadcast_to([128, T19, T19])
                    v.tensor_tensor(out=tmp3, in0=bsrc, in1=tr3, op=Alu.add)
                    v.drain()
                    v.tensor_reduce(out=red_a, in_=tmp3,
                                    axis=mybir.AxisListType.X, op=Alu.max)
                    v.drain()
                    if s < W:
                        dst = wb_a
                    else:
                        dst = store3[:, fslot if backward else s - W, :]
                    v.tensor_tensor(out=dst, in0=red_a, in1=fw3[:, fslot, :],
                                    op=Alu.add)
                    v.drain()

            scan(trf3, fwf3, fvs3, maskf_sb, initvf_sb, wbF_a, tmpF3, redF_a,
                 backward=False)
            scan(trb3, fwb3, gvs3, maskb_sb, initvb_sb, wbB_a, tmpB3, redB_a,
                 backward=True)

            v.tensor_tensor(out=A3, in0=fvs3[:, :, 0:INTER],
                            in1=gvs3[:, :, 0:INTER], op=Alu.add)
            v.drain()
            v.tensor_tensor(out=B3, in0=A3, in1=fwf3[:, W:NSTEP, 0:INTER],
                            op=Alu.subtract)
            v.drain()
            v.tensor_reduce(out=mx_a, in_=B3, axis=mybir.AxisListType.X,
                            op=Alu.max)
            v.drain()
            mxb = mx_a.unsqueeze(2).broadcast_to([128, L, INTER])
            v.tensor_tensor(out=K3, in0=B3, in1=mxb, op=Alu.is_ge)
            v.drain()
            iob = iota_sb.unsqueeze(1).broadcast_to([128, L, INTER])
            v.scalar_tensor_tensor(out=K3, in0=K3, scalar=-1000.0,
                                   in1=iob, op0=Alu.mult, op1=Alu.add)
            v.drain()
            v.tensor_reduce(out=pf_a, in_=K3, axis=mybir.AxisListType.X,
                            op=Alu.min)
            v.drain()
            pfb = pf_a.unsqueeze(2).broadcast_to([128, L, INTER])
            v.tensor_tensor(out=K3, in0=K3, in1=pfb, op=Alu.is_equal)
            v.drain()
            v.tensor_tensor(out=K3, in0=K3, in1=fwf3[:, W:NSTEP, 0:INTER],
                            op=Alu.mult)
            v.drain()
            v.tensor_reduce(out=fem_a, in_=K3, axis=mybir.AxisListType.X,
                            op=Alu.add)
            v.drain()
            v.tensor_copy(out=pi_a, in_=pf_a)
            v.drain()
            v.nop().then_inc(vD, 1)

    ctx.close()
    return nc


def _host_inputs(x, w_ih_f, b_ih_f, b_hh_f, w_ih_b, b_ih_b, b_hh_b,
                 w_tag, b_tag, transitions):
    sel = np.r_[0:HALF, 2 * HALF:4 * HALF]          # i, g, o rows
    Wf = np.ascontiguousarray(w_ih_f[sel].T)        # [768, 768]
    Wb = np.ascontiguousarray(w_ih_b[sel].T)
    wcat = np.concatenate([Wf, Wb], axis=1).astype(bf16)   # [768, 1536]
    wtagT = np.ascontiguousarray(w_tag.T).astype(bf16)     # [512, 20]

    # packed bf16 weights [128, 9296]: 6 wcat k-tiles then 4 wtagT k-tiles
    wbf = np.zeros((128, 9296), bf16)
    for k in range(6):
        wbf[:, k * 1536:(k + 1) * 1536] = wcat[k * 128:(k + 1) * 128, :]
    for k in range(4):
        wbf[:, 9216 + k * T: 9216 + (k + 1) * T] = wtagT[k * 128:(k + 1) * 128, :]

    bfv = (b_ih_f + b_hh_f)[sel]
    bbv = (b_ih_b + b_hh_b)[sel]
    bfull = np.concatenate([bfv, bbv]).astype(np.float32)    # [1536]
    bcat = np.zeros((128, 12), np.float32)
    for g in range(12):
        bcat[:, g] = bfull[g * 128:(g + 1) * 128]

    ob2 = np.zeros((2, 148), bf16)
    ob2[:, 0:128] = 1.0
    btag_hi = b_tag.astype(np.float32).astype(bf16)
    btag_lo = (b_tag.astype(np.float32) - btag_hi.astype(np.float32)).astype(bf16)
    ob2[0, 128:148] = btag_hi
    ob2[1, 128:148] = btag_lo

    tr = transitions.astype(np.float32)
    Sf = list(range(INTER)) + [START]
    Sb = list(range(INTER)) + [STOP]
    TRf = tr[np.ix_(Sf, Sf)]
    TRb = tr[np.ix_(Sb, Sb)].T

    init19 = np.full(T19, NEG, np.float32)
    init19[INTER] = 0.0

    toks = np.asarray(x, np.float32)[0, 1:S - 1]            # [16384, 768]

    in_maps = []
    for m in range(NCORES):
        f32c = np.zeros((128, 792), np.float32)
        f32c[:, 0:361] = TRf.reshape(1, -1)
        f32c[:, 361:722] = TRb.reshape(1, -1)
        f32c[:, 722] = 1.0                     # maskf
        f32c[:, 742] = 1.0                     # maskb
        f32c[:, 762:780] = np.arange(INTER) + 1000.0
        f32c[:, 780:792] = bcat
        if m == 0:
            f32c[0, 722] = 0.0
            f32c[0, 723:742] = init19
        if m == NCORES - 1:
            f32c[127, 742] = 0.0
            f32c[127, 743:762] = init19

        lo = m * NPC - PAD
        hi = m * NPC + NPC + PAD
        xsl = np.zeros((LTP, H), np.float32)
        a, b_ = max(lo, 0), min(hi, N)
        xsl[a - lo: a - lo + (b_ - a)] = toks[a:b_]
        in_maps.append({
            "xb": xsl.astype(bf16),
            "f32c": f32c,
            "wbf": wbf,
            "ob2": ob2,
        })
    return in_maps


def _run(in_maps, trace=False):
    from concourse.bass_utils import run_bass_kernel_spmd
    if "nc" not in _CACHE:
        _CACHE["nc"] = _build_nc()
    nc = _CACHE["nc"]
    res = run_bass_kernel_spmd(nc, in_maps, list(range(NCORES)), trace=trace)
    return res


def kernel(x, w_ih_f, w_hh_f, b_ih_f, b_hh_f, w_ih_b, w_hh_b, b_ih_b, b_hh_b,
           w_tag, b_tag, transitions, _trace=False, _ret_results=False):
    in_maps = _host_inputs(np.asarray(x), np.asarray(w_ih_f), np.asarray(b_ih_f),
                           np.asarray(b_hh_f), np.asarray(w_ih_b),
                           np.asarray(b_ih_b), np.asarray(b_hh_b),
                           np.asarray(w_tag), np.asarray(b_tag),
                           np.asarray(transitions))
    res = _run(in_maps, trace=_trace)
    results = res.results

    path = np.concatenate(
        [np.asarray(results[m]["pout"]).reshape(-1) for m in range(NCORES)])
    femis = np.concatenate(
        [np.asarray(results[m]["femis"]).reshape(-1) for m in range(NCORES)])

    # host score: f32-faithful accumulation in reference op order
    tr = np.asarray(transitions, np.float32)
    p = path.astype(np.int64)
    incs = np.empty(2 * N, np.float32)
    incs[0] = tr[p[0], START]
    incs[1] = femis[0]
    incs[2::2] = tr[p[1:], p[:-1]]
    incs[3::2] = femis[1:]
    s = np.add.accumulate(incs, dtype=np.float32)[-1]
    score = np.float32(s + tr[STOP, p[-1]])

    out = (np.float32(score), path.astype(np.int32))
    if _ret_results:
        return out, res
    return out
